# revision 11
# baseline (speedup 1.0000x reference)
"""Trainium2 Bass kernel for nn_CAM_58291296141385 (dense_mlp).

Data-parallel across 8 NeuronCores: each core processes 62500 rows
(padded to 123 tiles x 512 rows). Weights (<0.5 MB) are replicated.

Per 512-row tile, on each core (feature-major compute layout):
  - DMA-cast loads (fp32 DRAM -> bf16 SBUF), batch-major [128, 4, F]
  - L2 norms per row: square+accumulate on DVE, rsqrt via bit-hack+Newton
  - scale rows by 1/norm (per-partition scalar) while packing into
    [f1n | f2n] = 256 features = two 128-feature blocks X, Y
  - PE transposes to feature-major [128 feats, 512 rows] bf16
  - matmul chain: encoders -> affine attention -> tanh gate -> hidden relu
    -> collapsed regressor (r1/r2 fused into a single [*,1] vector on host)
"""

import numpy as np

import concourse.bass as bass
import concourse.bacc as bacc
import concourse.mybir as mybir
from concourse.tile import TileContext
from concourse.bass_utils import run_bass_kernel_spmd

dt = mybir.dt
AF = mybir.ActivationFunctionType
ALU = mybir.AluOpType

P = 128
BT = 512          # batch columns per tile
GRP = 4           # groups of 128 rows per tile
F1 = 88
F2 = 168
NCORES = 8
B = 500_000
ROWS_CORE = B // NCORES          # 62500
NT_FULL = (ROWS_CORE + BT - 1) // BT   # 123
PAD_ROWS = NT_FULL * BT                # 62976

_W_SPECS = [
    ("lx_aud", (P, P), dt.bfloat16),
    ("lx_vis", (P, P), dt.bfloat16),
    ("ly_vis", (P, P), dt.bfloat16),
    ("AaT1", (P, P), dt.bfloat16),
    ("AaT2", (P, P), dt.bfloat16),
    ("AvT1", (P, P), dt.bfloat16),
    ("AvT2", (P, P), dt.bfloat16),
    ("wcaT", (P, 32), dt.bfloat16),
    ("waT", (P, 32), dt.bfloat16),
    ("wcvT", (P, 32), dt.bfloat16),
    ("wvT", (P, 32), dt.bfloat16),
    ("u_av", (64, 1), dt.bfloat16),
    ("g_a", (P, 1), dt.bfloat16),
    ("g_v", (P, 1), dt.bfloat16),
    ("e1b", (P, 1), dt.float32),
    ("e2b", (P, 1), dt.float32),
    ("ident", (P, P), dt.bfloat16),
]

_NC_CACHE = {}


def build_program(nt: int) -> bass.Bass:
    if nt in _NC_CACHE:
        return _NC_CACHE[nt]
    nc = bacc.Bacc("TRN2", target_bir_lowering=False, debug=False)
    n = nt * BT
    f1 = nc.declare_dram_parameter("f1", [n, F1], dt.float32, isOutput=False)
    f2 = nc.declare_dram_parameter("f2", [n, F2], dt.float32, isOutput=False)
    wtot = sum(int(np.prod(s)) for _, s, _ in _W_SPECS)
    wpk = nc.declare_dram_parameter("wpk", [wtot], dt.float32, isOutput=False)
    out = nc.declare_dram_parameter("out", [n], dt.float32, isOutput=True)

    f1r = f1.rearrange("(n g p) f -> n p g f", g=GRP, p=P)
    f2r = f2.rearrange("(n g p) f -> n p g f", g=GRP, p=P)
    outr = out.rearrange("(n o f) -> n o f", o=1, f=BT)

    with TileContext(nc) as tc:
        with (
            tc.tile_pool(name="const", bufs=1) as cpool,
            tc.tile_pool(name="io", bufs=3) as iop,
            tc.tile_pool(name="work", bufs=2) as wp,
            tc.tile_pool(name="pst", bufs=3, space="PSUM") as ptp,
            tc.tile_pool(name="psc", bufs=1, space="PSUM") as pcp,
        ):
            # --- one-time: load weights (fp32 DRAM -> SBUF, cast where bf16)
            W = {}
            off = 0
            for name, shp, dtype in _W_SPECS:
                sz = int(np.prod(shp))
                t = cpool.tile(list(shp), dtype, name=name, tag=name)
                nc.gpsimd.dma_start(
                    out=t[:],
                    in_=wpk[off:off + sz].rearrange("(p f) -> p f", f=shp[1]),
                )
                W[name] = t
                off += sz

            for ti in range(nt):
                # --- load (cast fp32 -> bf16), batch-major interleaved groups
                f1t = iop.tile([P, GRP, F1], dt.bfloat16, name="f1t", tag="f1t")
                f2t = iop.tile([P, GRP, F2], dt.bfloat16, name="f2t", tag="f2t")
                nc.gpsimd.dma_start(out=f1t[:], in_=f1r[ti])
                nc.gpsimd.dma_start(out=f2t[:], in_=f2r[ti])

                # --- row sums of squares -> ss[:, 0:4]=f1 groups, 4:8=f2.
                # Disjoint scratch slices per STT: the S2S2D2_STT ISA struct
                # supports only ONE sync-wait, so no same-engine WAW sems.
                sq1 = wp.tile([P, GRP, F1], dt.bfloat16, name="sq1", tag="sq1")
                sq2 = wp.tile([P, GRP, F2], dt.bfloat16, name="sq2", tag="sq2")
                ss = wp.tile([P, 8], dt.float32, name="ss", tag="ss")
                for g in range(GRP):
                    nc.vector.scalar_tensor_tensor(
                        out=sq1[:, g, :], in0=f1t[:, g, :], scalar=0.0,
                        in1=f1t[:, g, :], op0=ALU.bypass, op1=ALU.mult,
                        accum_out=ss[:, g:g + 1])
                    nc.vector.scalar_tensor_tensor(
                        out=sq2[:, g, :], in0=f2t[:, g, :], scalar=0.0,
                        in1=f2t[:, g, :], op0=ALU.bypass, op1=ALU.mult,
                        accum_out=ss[:, 4 + g:5 + g])

                # --- rn = 1/sqrt(max(ss, 1e-24)) via bit hack + 2 Newton
                # iterations (DVE).  max(ss,1e-24) == the reference's
                # x / max(sqrt(ss), 1e-12) for all non-degenerate rows.
                nc.vector.tensor_scalar_max(ss[:], ss[:], 1e-24)
                ssi = ss.bitcast(dt.int32)
                y0i = wp.tile([P, 8], dt.int32, name="y0i", tag="y0i")
                nc.vector.tensor_scalar(
                    out=y0i[:], in0=ssi, scalar1=1, scalar2=None,
                    op0=ALU.arith_shift_right)
                # magic - z == (z ^ -1) + (magic + 1); walrus forbids mixing
                # bitwise and arith ops in one tensor_scalar, so two steps.
                nc.vector.tensor_scalar(
                    out=y0i[:], in0=y0i[:], scalar1=-1, scalar2=None,
                    op0=ALU.bitwise_xor)
                nc.vector.tensor_scalar(
                    out=y0i[:], in0=y0i[:], scalar1=0x5F3759E0, scalar2=None,
                    op0=ALU.add)
                ya = y0i.bitcast(dt.float32)
                t1 = wp.tile([P, 8], dt.float32, name="t1", tag="t1")
                yb = wp.tile([P, 8], dt.float32, name="yb", tag="yb")
                for cur, nxt in ((ya, yb[:]), (yb[:], ya)):
                    nc.vector.scalar_tensor_tensor(
                        out=t1[:], in0=cur, scalar=0.0, in1=cur,
                        op0=ALU.bypass, op1=ALU.mult)
                    nc.vector.scalar_tensor_tensor(
                        out=t1[:], in0=t1[:], scalar=-0.5, in1=ss[:],
                        op0=ALU.mult, op1=ALU.mult)
                    nc.vector.scalar_tensor_tensor(
                        out=nxt, in0=t1[:], scalar=1.5, in1=cur,
                        op0=ALU.add, op1=ALU.mult)
                rn = ya

                # --- normalize + pack: xy[:, g, 0:88]=f1n, [:, g, 88:256]=f2n
                xy = wp.tile([P, GRP, 256], dt.bfloat16, name="xy", tag="xy")
                for g in range(GRP):
                    nc.vector.tensor_scalar_mul(
                        xy[:, g, 0:F1], f1t[:, g, :], rn[:, g:g + 1])
                    nc.vector.tensor_scalar_mul(
                        xy[:, g, F1:256], f2t[:, g, :], rn[:, 4 + g:5 + g])

                # --- transpose to feature-major via PE (2 groups per bank)
                xt = wp.tile([P, BT], dt.bfloat16, name="xt", tag="xt")
                yt = wp.tile([P, BT], dt.bfloat16, name="yt", tag="yt")
                for h in range(2):
                    tp = ptp.tile([P, BT], dt.bfloat16, name="tp", tag="tp")
                    for k in range(2):
                        g = 2 * h + k
                        nc.tensor.transpose(
                            tp[:, k * P:(k + 1) * P], xy[:, g, 0:P],
                            W["ident"][:])
                        nc.tensor.transpose(
                            tp[:, 256 + k * P:256 + (k + 1) * P],
                            xy[:, g, P:256], W["ident"][:])
                    nc.vector.tensor_copy(
                        xt[:, h * 256:(h + 1) * 256], tp[:, 0:256])
                    nc.scalar.activation(
                        yt[:, h * 256:(h + 1) * 256], tp[:, 256:512], AF.Copy)

                # --- encoders: aud = E1 @ f1n + b1 ; vis = E2 @ f2n + b2
                audp = pcp.tile([P, BT], dt.float32, name="audp", tag="audp")
                visp = pcp.tile([P, BT], dt.float32, name="visp", tag="visp")
                nc.tensor.matmul(audp[:], W["lx_aud"][:], xt[:],
                                 start=True, stop=True)
                nc.tensor.matmul(visp[:], W["lx_vis"][:], xt[:],
                                 start=True, stop=False)
                nc.tensor.matmul(visp[:], W["ly_vis"][:], yt[:],
                                 start=False, stop=True)
                audT = wp.tile([P, BT], dt.bfloat16, name="audT", tag="audT")
                visT = wp.tile([P, BT], dt.bfloat16, name="visT", tag="visT")
                nc.vector.tensor_scalar_add(audT[:], audp[:], W["e1b"][:])
                nc.scalar.activation(visT[:], visp[:], AF.Identity,
                                     bias=W["e2b"][:])

                # --- affine attention: a_t, v_t
                atp = pcp.tile([P, BT], dt.float32, name="atp", tag="atp")
                vtp = pcp.tile([P, BT], dt.float32, name="vtp", tag="vtp")
                nc.tensor.matmul(atp[:], W["AaT1"][:], audT[:],
                                 start=True, stop=False)
                nc.tensor.matmul(atp[:], W["AaT2"][:], visT[:],
                                 start=False, stop=True)
                nc.tensor.matmul(vtp[:], W["AvT1"][:], audT[:],
                                 start=True, stop=False)
                nc.tensor.matmul(vtp[:], W["AvT2"][:], visT[:],
                                 start=False, stop=True)

                # --- att = tanh(aud * a_t / 16), vis likewise
                tmp = wp.tile([P, 2, BT], dt.bfloat16, name="tmp", tag="tmp")
                nc.vector.tensor_mul(tmp[:, 0, :], atp[:], audT[:])
                nc.vector.tensor_mul(tmp[:, 1, :], vtp[:], visT[:])
                att = wp.tile([P, 2, BT], dt.bfloat16, name="att", tag="att")
                nc.scalar.activation(att[:], tmp[:], AF.Tanh,
                                     scale=1.0 / 16.0)

                # --- hidden: h_a rows 0:32, h_v rows 32:64, out row 64
                hp = pcp.tile([P, BT], dt.float32, name="hp", tag="hp")
                nc.tensor.matmul(hp[0:32, :], W["wcaT"][:], att[:, 0, :],
                                 start=True, stop=False)
                nc.tensor.matmul(hp[0:32, :], W["waT"][:], audT[:],
                                 start=False, stop=True)
                nc.tensor.matmul(hp[32:64, :], W["wcvT"][:], att[:, 1, :],
                                 start=True, stop=False)
                nc.tensor.matmul(hp[32:64, :], W["wvT"][:], visT[:],
                                 start=False, stop=True)
                hsb = wp.tile([64, BT], dt.bfloat16, name="hsb", tag="hsb")
                nc.vector.tensor_scalar_max(hsb[0:32, :], hp[0:32, :], 0.0)
                nc.scalar.activation(hsb[32:64, :], hp[32:64, :], AF.Relu)

                # --- collapsed regressor -> psum row 64
                nc.tensor.matmul(hp[64:65, :], W["u_av"][:], hsb[:],
                                 start=True, stop=False)
                nc.tensor.matmul(hp[64:65, :], W["g_a"][:], audT[:],
                                 start=False, stop=False)
                nc.tensor.matmul(hp[64:65, :], W["g_v"][:], visT[:],
                                 start=False, stop=True)
                osb = iop.tile([1, BT], dt.float32, name="osb", tag="osb")
                nc.scalar.activation(osb[:], hp[64:65, :], AF.Copy)
                nc.sync.dma_start(out=outr[ti], in_=osb[:])

    nc.compile()
    _NC_CACHE[nt] = nc
    return nc


def prep_weights(i):
    """Host-side weight fusion. Returns (flat fp32 pack, scalar out offset)."""
    f32 = np.float32
    e1w = np.asarray(i["e1_w"], f32)     # [128, 88]
    e1b = np.asarray(i["e1_b"], f32)     # [128]
    e2w = np.asarray(i["e2_w"], f32)     # [128, 168]
    e2b = np.asarray(i["e2_b"], f32)
    aff_a = np.asarray(i["aff_a"], f32)  # [128, 256]
    aff_v = np.asarray(i["aff_v"], f32)
    w_a = np.asarray(i["w_a"], f32)      # [32, 128]
    w_v = np.asarray(i["w_v"], f32)
    w_ca = np.asarray(i["w_ca"], f32)
    w_cv = np.asarray(i["w_cv"], f32)
    w_ha = np.asarray(i["w_ha"], f32)    # [8, 32]
    w_hv = np.asarray(i["w_hv"], f32)
    e3w = np.asarray(i["e3_w"], f32)     # [8, 128]
    e3b = np.asarray(i["e3_b"], f32)
    e4w = np.asarray(i["e4_w"], f32)
    e4b = np.asarray(i["e4_b"], f32)
    r1w = np.asarray(i["r1_w"], f32)     # [128, 16]
    r1b = np.asarray(i["r1_b"], f32)     # [128]
    r2w = np.asarray(i["r2_w"], f32)     # [1, 128]
    r2b = np.asarray(i["r2_b"], f32)     # [1]

    vals = {}
    lx_aud = np.zeros((P, P), f32)
    lx_aud[0:F1, :] = e1w.T                 # X partitions 0:88 = f1 feats
    vals["lx_aud"] = lx_aud
    lx_vis = np.zeros((P, P), f32)
    lx_vis[F1:P, :] = e2w.T[0:40, :]        # X partitions 88:128 = f2 feats 0:40
    vals["lx_vis"] = lx_vis
    vals["ly_vis"] = np.ascontiguousarray(e2w.T[40:168, :])  # Y = f2 feats 40:168
    vals["AaT1"] = np.ascontiguousarray(aff_a[:, :128].T)
    vals["AaT2"] = np.ascontiguousarray(aff_a[:, 128:].T)
    vals["AvT1"] = np.ascontiguousarray(aff_v[:, :128].T)
    vals["AvT2"] = np.ascontiguousarray(aff_v[:, 128:].T)
    vals["wcaT"] = np.ascontiguousarray(w_ca.T)
    vals["waT"] = np.ascontiguousarray(w_a.T)
    vals["wcvT"] = np.ascontiguousarray(w_cv.T)
    vals["wvT"] = np.ascontiguousarray(w_v.T)

    R = (r1w.T @ r2w.T).reshape(16)         # collapsed regressor
    Ra, Rv = R[:8], R[8:]
    u_av = np.zeros((64, 1), f32)
    u_av[0:32, 0] = w_ha.T @ Ra
    u_av[32:64, 0] = w_hv.T @ Rv
    vals["u_av"] = u_av
    vals["g_a"] = (e3w.T @ Ra).reshape(P, 1)
    vals["g_v"] = (e4w.T @ Rv).reshape(P, 1)
    vals["e1b"] = e1b.reshape(P, 1)
    vals["e2b"] = e2b.reshape(P, 1)
    vals["ident"] = np.eye(P, dtype=f32)

    const = float(e3b @ Ra + e4b @ Rv + float(r1b @ r2w[0]) + float(r2b[0]))
    pack = np.concatenate(
        [np.ascontiguousarray(vals[name]).reshape(-1) for name, _, _ in _W_SPECS])
    return pack.astype(f32), const


def kernel(**inputs) -> np.ndarray:
    f1 = np.asarray(inputs["f1"], np.float32)
    f2 = np.asarray(inputs["f2"], np.float32)
    b = f1.shape[0]
    rows_core = b // NCORES
    nt = (rows_core + BT - 1) // BT
    pad_rows = nt * BT

    wpk, const = prep_weights(inputs)
    nc = build_program(nt)

    in_maps = []
    for c in range(NCORES):
        f1c = np.zeros((pad_rows, F1), np.float32)
        f2c = np.zeros((pad_rows, F2), np.float32)
        f1c[:rows_core] = f1[c * rows_core:(c + 1) * rows_core]
        f2c[:rows_core] = f2[c * rows_core:(c + 1) * rows_core]
        in_maps.append({"f1": f1c, "f2": f2c, "wpk": wpk})

    res = run_bass_kernel_spmd(nc, in_maps, list(range(NCORES))).results

    out = np.empty((b, 1), np.float32)
    for c in range(NCORES):
        oc = np.asarray(res[c]["out"], np.float32).reshape(-1)
        out[c * rows_core:(c + 1) * rows_core, 0] = oc[:rows_core]
    out += np.float32(const)
    return out


# revision 31
# speedup vs baseline: 144.1351x; 144.1351x over previous
"""Trainium2 Bass kernel for nn_CAM_58291296141385 (dense_mlp).

Data-parallel across 8 NeuronCores: each core processes 62500 rows
(padded to 62 tiles x 1024 rows). Weights (<0.5 MB) are replicated.

Per 1024-row tile, on each core (feature-major compute layout):
  - DMA-cast loads (fp32 DRAM -> bf16 SBUF), batch-major [128, 8, F]
  - L2 norms per row: square+accumulate STTs on DVE, rsqrt via
    bit-hack + Newton (all DVE; avoids ACT table-set thrash with Tanh)
  - scale rows by 1/norm (per-partition scalar) while packing into
    [f1n | f2n] = 256 features = two 128-feature blocks X, Y
  - PE transposes to feature-major bf16 (4 groups per PSUM bank)
  - matmul chain: encoders -> affine attention -> tanh gate -> hidden relu
    -> collapsed regressor (r1/r2 fused into a single [*,1] vector on host)
"""

import numpy as np

import concourse.bass as bass
import concourse.bacc as bacc
import concourse.mybir as mybir
from concourse.tile import TileContext
from concourse.bass_utils import run_bass_kernel_spmd

dt = mybir.dt
AF = mybir.ActivationFunctionType
ALU = mybir.AluOpType

P = 128
BT = 1024         # batch columns per tile
GRP = 8           # groups of 128 rows per tile
HB = 512          # matmul moving-dim half (PSUM bank limit)
F1 = 88
F2 = 168
NCORES = 8
B = 500_000
ROWS_CORE = B // NCORES                 # 62500
NT_FULL = (ROWS_CORE + BT - 1) // BT    # 62
PAD_ROWS = NT_FULL * BT                 # 63488

_W_SPECS = [
    ("lx_aud", (P, P), dt.bfloat16),
    ("lx_vis", (P, P), dt.bfloat16),
    ("ly_vis", (P, P), dt.bfloat16),
    ("AaT1", (P, P), dt.bfloat16),
    ("AaT2", (P, P), dt.bfloat16),
    ("AvT1", (P, P), dt.bfloat16),
    ("AvT2", (P, P), dt.bfloat16),
    ("wcaT", (P, 32), dt.bfloat16),
    ("waT", (P, 32), dt.bfloat16),
    ("wcvT", (P, 32), dt.bfloat16),
    ("wvT", (P, 32), dt.bfloat16),
    ("u_av", (64, 1), dt.bfloat16),
    ("g_a", (P, 1), dt.bfloat16),
    ("g_v", (P, 1), dt.bfloat16),
    ("e1b", (P, 1), dt.float32),
    ("e2b", (P, 1), dt.float32),
    ("ident", (P, P), dt.bfloat16),
]

_NC_CACHE = {}


def build_program(nt: int, repeat: int = 1) -> bass.Bass:
    key = (nt, repeat)
    if key in _NC_CACHE:
        return _NC_CACHE[key]
    nc = bacc.Bacc("TRN2", target_bir_lowering=False, debug=False)
    n = nt * BT
    # fc = [f1 | f2] concatenated on host -> one DMA per tile with
    # per-partition-contiguous chunks (row r = p*GRP + g).
    fc = nc.declare_dram_parameter("fc", [n, 256], dt.float32, isOutput=False)
    wtot = sum(int(np.prod(s)) for _, s, _ in _W_SPECS)
    wpk = nc.declare_dram_parameter("wpk", [wtot], dt.float32, isOutput=False)
    out = nc.declare_dram_parameter("out", [n], dt.float32, isOutput=True)

    fcr = fc.rearrange("(n p g) f -> n p g f", p=P, g=GRP)
    outr = out.rearrange("(n o f) -> n o f", o=1, f=BT)

    with TileContext(nc) as tc:
        with (
            tc.tile_pool(name="const", bufs=1) as cpool,
            tc.tile_pool(name="io", bufs=4) as iop,
            tc.tile_pool(name="work", bufs=3) as wp,
            tc.tile_pool(name="pst", bufs=2, space="PSUM") as ptp,
            tc.tile_pool(name="psc", bufs=1, space="PSUM") as pcp,
        ):
            # --- one-time: load weights (fp32 DRAM -> SBUF, cast where bf16)
            W = {}
            off = 0
            for name, shp, dtype in _W_SPECS:
                sz = int(np.prod(shp))
                t = cpool.tile(list(shp), dtype, name=name, tag=name)
                nc.gpsimd.dma_start(
                    out=t[:],
                    in_=wpk[off:off + sz].rearrange("(p f) -> p f", f=shp[1]),
                )
                W[name] = t
                off += sz

            import contextlib
            rep_ctx = tc.For_i(0, repeat, 1) if repeat > 1 else (
                contextlib.nullcontext())
            with rep_ctx:
              for ti in range(nt):
                # --- load (cast fp32 -> bf16), per-partition contiguous rows
                fct = iop.tile([P, GRP, 256], dt.bfloat16, name="fct", tag="fct")
                nc.gpsimd.dma_start(out=fct[:], in_=fcr[ti])
                f1t = fct[:, :, 0:F1]
                f2t = fct[:, :, F1:256]

                # --- row sums of squares -> ss[:, 0:8]=f1, 8:16=f2.
                # Disjoint scratch per STT (single-wait ISA struct).
                sq1 = wp.tile([P, GRP, F1], dt.bfloat16, name="sq1", tag="sq1")
                sq2 = wp.tile([P, GRP, F2], dt.bfloat16, name="sq2", tag="sq2")
                ss = wp.tile([P, 16], dt.float32, name="ss", tag="ss")
                for g in range(GRP):
                    nc.vector.scalar_tensor_tensor(
                        out=sq1[:, g, :], in0=f1t[:, g, :], scalar=0.0,
                        in1=f1t[:, g, :], op0=ALU.bypass, op1=ALU.mult,
                        accum_out=ss[:, g:g + 1])
                    nc.vector.scalar_tensor_tensor(
                        out=sq2[:, g, :], in0=f2t[:, g, :], scalar=0.0,
                        in1=f2t[:, g, :], op0=ALU.bypass, op1=ALU.mult,
                        accum_out=ss[:, GRP + g:GRP + g + 1])

                # --- rn = 1/sqrt(max(ss, 1e-24)) via bit hack + Newton (DVE).
                # max(ss,1e-24) == the reference's x / max(sqrt(ss), 1e-12).
                nc.vector.tensor_scalar_max(ss[:], ss[:], 1e-24)
                ssi = ss.bitcast(dt.int32)
                y0i = wp.tile([P, 16], dt.int32, name="y0i", tag="y0i")
                nc.vector.tensor_scalar(
                    out=y0i[:], in0=ssi, scalar1=1, scalar2=None,
                    op0=ALU.arith_shift_right)
                # magic - z == (z ^ -1) + (magic + 1); walrus forbids mixing
                # bitwise and arith ops in one tensor_scalar, so two steps.
                nc.vector.tensor_scalar(
                    out=y0i[:], in0=y0i[:], scalar1=-1, scalar2=None,
                    op0=ALU.bitwise_xor)
                nc.vector.tensor_scalar(
                    out=y0i[:], in0=y0i[:], scalar1=0x5F3759E0, scalar2=None,
                    op0=ALU.add)
                ya = y0i.bitcast(dt.float32)
                t1 = wp.tile([P, 16], dt.float32, name="t1", tag="t1")
                yb = wp.tile([P, 16], dt.float32, name="yb", tag="yb")
                nc.vector.scalar_tensor_tensor(
                    out=t1[:], in0=ya, scalar=0.0, in1=ya,
                    op0=ALU.bypass, op1=ALU.mult)
                nc.vector.scalar_tensor_tensor(
                    out=t1[:], in0=t1[:], scalar=-0.5, in1=ss[:],
                    op0=ALU.mult, op1=ALU.mult)
                nc.vector.scalar_tensor_tensor(
                    out=yb[:], in0=t1[:], scalar=1.5, in1=ya,
                    op0=ALU.add, op1=ALU.mult)
                rn = yb[:]

                # --- normalize + pack: xy[:, g, 0:88]=f1n, [:, g, 88:256]=f2n
                # scale on GPSIMD (1-input SBUF->SBUF, keeps DVE free)
                xy = wp.tile([P, GRP, 256], dt.bfloat16, name="xy", tag="xy")
                for g in range(GRP):
                    nc.vector.tensor_scalar_mul(
                        xy[:, g, 0:F1], f1t[:, g, :], rn[:, g:g + 1])
                    nc.vector.tensor_scalar_mul(
                        xy[:, g, F1:256], f2t[:, g, :],
                        rn[:, GRP + g:GRP + g + 1])

                # --- transpose to feature-major via PE (4 groups per bank):
                # tp cols 0:512 = X parts, 512:1024 = Y parts
                xt = wp.tile([P, BT], dt.bfloat16, name="xt", tag="xt")
                yt = wp.tile([P, BT], dt.bfloat16, name="yt", tag="yt")
                for h in range(2):
                    tp = ptp.tile([P, BT], dt.bfloat16, name="tp", tag="tp")
                    for k in range(4):
                        g = 4 * h + k
                        nc.tensor.transpose(
                            tp[:, k * P:(k + 1) * P], xy[:, g, 0:P],
                            W["ident"][:])
                        nc.tensor.transpose(
                            tp[:, 512 + k * P:512 + (k + 1) * P],
                            xy[:, g, P:256], W["ident"][:])
                    nc.vector.tensor_copy(
                        xt[:, h * 512:(h + 1) * 512], tp[:, 0:512])
                    nc.scalar.activation(
                        yt[:, h * 512:(h + 1) * 512], tp[:, 512:1024], AF.Copy)

                # --- encoders: aud = E1 @ f1n + b1 ; vis = E2 @ f2n + b2
                audp = pcp.tile([P, BT], dt.float32, name="audp", tag="psA")
                visp = pcp.tile([P, BT], dt.float32, name="visp", tag="psB")
                for h in range(2):
                    s = slice(h * HB, (h + 1) * HB)
                    nc.tensor.matmul(audp[:, s], W["lx_aud"][:], xt[:, s],
                                     start=True, stop=True)
                    nc.tensor.matmul(visp[:, s], W["lx_vis"][:], xt[:, s],
                                     start=True, stop=False)
                    nc.tensor.matmul(visp[:, s], W["ly_vis"][:], yt[:, s],
                                     start=False, stop=True)
                audT = wp.tile([P, BT], dt.bfloat16, name="audT", tag="audT")
                visT = wp.tile([P, BT], dt.bfloat16, name="visT", tag="visT")
                nc.scalar.activation(audT[:], audp[:], AF.Identity,
                                     bias=W["e1b"][:])
                nc.scalar.activation(visT[:], visp[:], AF.Identity,
                                     bias=W["e2b"][:])

                # --- affine attention: a_t, v_t (reuse the e-layer PSUM tags)
                atp = pcp.tile([P, BT], dt.float32, name="atp", tag="psA")
                vtp = pcp.tile([P, BT], dt.float32, name="vtp", tag="psB")
                for h in range(2):
                    s = slice(h * HB, (h + 1) * HB)
                    nc.tensor.matmul(atp[:, s], W["AaT1"][:], audT[:, s],
                                     start=True, stop=False)
                    nc.tensor.matmul(atp[:, s], W["AaT2"][:], visT[:, s],
                                     start=False, stop=True)
                    nc.tensor.matmul(vtp[:, s], W["AvT1"][:], audT[:, s],
                                     start=True, stop=False)
                    nc.tensor.matmul(vtp[:, s], W["AvT2"][:], visT[:, s],
                                     start=False, stop=True)

                # --- att = tanh(aud * a_t / 16), vis likewise
                tmp = wp.tile([P, 2, BT], dt.bfloat16, name="tmp", tag="tmp")
                nc.vector.tensor_mul(tmp[:, 0, :], atp[:], audT[:])
                nc.vector.tensor_mul(tmp[:, 1, :], vtp[:], visT[:])
                att = wp.tile([P, 2, BT], dt.bfloat16, name="att", tag="att")
                nc.scalar.activation(att[:], tmp[:], AF.Tanh,
                                     scale=1.0 / 16.0)

                # --- hidden: h_a rows 0:32, h_v rows 32:64, out row 64
                hp = pcp.tile([P, BT], dt.float32, name="hp", tag="psH")
                for h in range(2):
                    s = slice(h * HB, (h + 1) * HB)
                    nc.tensor.matmul(hp[0:32, s], W["wcaT"][:], att[:, 0, s],
                                     start=True, stop=False)
                    nc.tensor.matmul(hp[0:32, s], W["waT"][:], audT[:, s],
                                     start=False, stop=True)
                    nc.tensor.matmul(hp[32:64, s], W["wcvT"][:], att[:, 1, s],
                                     start=True, stop=False)
                    nc.tensor.matmul(hp[32:64, s], W["wvT"][:], visT[:, s],
                                     start=False, stop=True)
                hsb = wp.tile([64, BT], dt.bfloat16, name="hsb", tag="hsb")
                nc.scalar.activation(hsb[:, :], hp[0:64, :], AF.Relu)

                # --- collapsed regressor -> psum row 64
                for h in range(2):
                    s = slice(h * HB, (h + 1) * HB)
                    nc.tensor.matmul(hp[64:65, s], W["u_av"][:], hsb[:, s],
                                     start=True, stop=False)
                    nc.tensor.matmul(hp[64:65, s], W["g_a"][:], audT[:, s],
                                     start=False, stop=False)
                    nc.tensor.matmul(hp[64:65, s], W["g_v"][:], visT[:, s],
                                     start=False, stop=True)
                osb = iop.tile([1, BT], dt.float32, name="osb", tag="osb")
                nc.scalar.activation(osb[:], hp[64:65, :], AF.Copy)
                nc.sync.dma_start(out=outr[ti], in_=osb[:])

    nc.compile()
    _NC_CACHE[key] = nc
    return nc


def prep_weights(i):
    """Host-side weight fusion. Returns (flat fp32 pack, scalar out offset)."""
    f32 = np.float32
    e1w = np.asarray(i["e1_w"], f32)     # [128, 88]
    e1b = np.asarray(i["e1_b"], f32)     # [128]
    e2w = np.asarray(i["e2_w"], f32)     # [128, 168]
    e2b = np.asarray(i["e2_b"], f32)
    aff_a = np.asarray(i["aff_a"], f32)  # [128, 256]
    aff_v = np.asarray(i["aff_v"], f32)
    w_a = np.asarray(i["w_a"], f32)      # [32, 128]
    w_v = np.asarray(i["w_v"], f32)
    w_ca = np.asarray(i["w_ca"], f32)
    w_cv = np.asarray(i["w_cv"], f32)
    w_ha = np.asarray(i["w_ha"], f32)    # [8, 32]
    w_hv = np.asarray(i["w_hv"], f32)
    e3w = np.asarray(i["e3_w"], f32)     # [8, 128]
    e3b = np.asarray(i["e3_b"], f32)
    e4w = np.asarray(i["e4_w"], f32)
    e4b = np.asarray(i["e4_b"], f32)
    r1w = np.asarray(i["r1_w"], f32)     # [128, 16]
    r1b = np.asarray(i["r1_b"], f32)     # [128]
    r2w = np.asarray(i["r2_w"], f32)     # [1, 128]
    r2b = np.asarray(i["r2_b"], f32)     # [1]

    vals = {}
    lx_aud = np.zeros((P, P), f32)
    lx_aud[0:F1, :] = e1w.T                 # X partitions 0:88 = f1 feats
    vals["lx_aud"] = lx_aud
    lx_vis = np.zeros((P, P), f32)
    lx_vis[F1:P, :] = e2w.T[0:40, :]        # X partitions 88:128 = f2 feats 0:40
    vals["lx_vis"] = lx_vis
    vals["ly_vis"] = np.ascontiguousarray(e2w.T[40:168, :])  # Y = f2 feats 40:168
    vals["AaT1"] = np.ascontiguousarray(aff_a[:, :128].T)
    vals["AaT2"] = np.ascontiguousarray(aff_a[:, 128:].T)
    vals["AvT1"] = np.ascontiguousarray(aff_v[:, :128].T)
    vals["AvT2"] = np.ascontiguousarray(aff_v[:, 128:].T)
    vals["wcaT"] = np.ascontiguousarray(w_ca.T)
    vals["waT"] = np.ascontiguousarray(w_a.T)
    vals["wcvT"] = np.ascontiguousarray(w_cv.T)
    vals["wvT"] = np.ascontiguousarray(w_v.T)

    R = (r1w.T @ r2w.T).reshape(16)         # collapsed regressor
    Ra, Rv = R[:8], R[8:]
    u_av = np.zeros((64, 1), f32)
    u_av[0:32, 0] = w_ha.T @ Ra
    u_av[32:64, 0] = w_hv.T @ Rv
    vals["u_av"] = u_av
    vals["g_a"] = (e3w.T @ Ra).reshape(P, 1)
    vals["g_v"] = (e4w.T @ Rv).reshape(P, 1)
    vals["e1b"] = e1b.reshape(P, 1)
    vals["e2b"] = e2b.reshape(P, 1)
    vals["ident"] = np.eye(P, dtype=f32)

    const = float(e3b @ Ra + e4b @ Rv + float(r1b @ r2w[0]) + float(r2b[0]))
    pack = np.concatenate(
        [np.ascontiguousarray(vals[name]).reshape(-1) for name, _, _ in _W_SPECS])
    return pack.astype(f32), const


def make_core_input(f1c_rows, f2c_rows, pad_rows):
    """Concat f1|f2 and zero-pad to pad_rows -> [pad_rows, 256] fp32."""
    rows = f1c_rows.shape[0]
    fc = np.zeros((pad_rows, 256), np.float32)
    fc[:rows, 0:F1] = f1c_rows
    fc[:rows, F1:256] = f2c_rows
    return fc


def unpermute_out(oc, nt):
    """Feature-major column order (g*128+p) -> row order (p*GRP+g)."""
    return oc.reshape(nt, GRP, P).transpose(0, 2, 1).reshape(-1)


def kernel(**inputs) -> np.ndarray:
    f1 = np.asarray(inputs["f1"], np.float32)
    f2 = np.asarray(inputs["f2"], np.float32)
    b = f1.shape[0]
    rows_core = b // NCORES
    nt = (rows_core + BT - 1) // BT
    pad_rows = nt * BT

    wpk, const = prep_weights(inputs)
    nc = build_program(nt)

    in_maps = []
    for c in range(NCORES):
        fc = make_core_input(f1[c * rows_core:(c + 1) * rows_core],
                             f2[c * rows_core:(c + 1) * rows_core], pad_rows)
        in_maps.append({"fc": fc, "wpk": wpk})

    res = run_bass_kernel_spmd(nc, in_maps, list(range(NCORES))).results

    out = np.empty((b, 1), np.float32)
    for c in range(NCORES):
        oc = np.asarray(res[c]["out"], np.float32).reshape(-1)
        out[c * rows_core:(c + 1) * rows_core, 0] = unpermute_out(
            oc, nt)[:rows_core]
    out += np.float32(const)
    return out
